# revision 31
# baseline (speedup 1.0000x reference)
"""BD3LM block-diffusion decoder layer on 8 trn2 NeuronCores.

Sharding: core = 2*b + g  (b = batch 0..3, g = head-group 0..1, 8 heads each).
Each core: QKV projections for its batch/head-group, sparse BD3LM attention
(only ~80 of 256 score tiles per head), O-projection against its Wo row-slice.
Host: sums the two group partials per batch and adds the (bv @ Wo + bo)
correction (softmax rows sum to 1, so the v-bias contributes exactly bv @ Wo).

All matmuls fp16 (1 cyc/row at every moving size vs f32r's 4 cyc/row under
256; fast weight load; half DMA; 2x DVE). fp32 PSUM accumulation throughout.

Every phase draws PSUM from ONE shared 4-slot [128,512] ring (+4 banks of
attention ctx accumulators) opened once per iteration - no pool transitions
between phases, so phase tails overlap the next phase's matmuls:
  V pass     v[t,ch] tiles with a per-head ones column (denominators free)
  for c in 0..3:
      QK(c)    weight-chunk stationary, 2 moving 512-slabs per load
      ATTN(c)  heads (2c,2c+1) on partitions 0-63/64-127: paired K=64 score
               matmuls issue adjacently -> concurrent PE row groups. Scores
               transposed [k,q]; exp on ACT (no max-subtraction; scores
               ~N(0,1)); per-q-bank normalize as soon as the bank completes.
  O-proj     ctxT chunk stationary, reused for both 512-col Wo halves
"""

import numpy as np

import concourse.bass as bass
import concourse.mybir as mybir
import concourse.tile as tile
from concourse import bacc
from concourse.bass_utils import run_bass_kernel_spmd

F32 = mybir.dt.float32
F16 = mybir.dt.float16
Act = mybir.ActivationFunctionType

B, T, D = 4, 2048, 1024
H, HD = 16, 64
L = T // 2           # 1024, length of each of [xt | x0]
BS = 4               # block size
G = 2                # head groups (cores per batch)
DG = D // G          # 512 channels per group
HG = H // G          # 8 heads per core
P = 128
NT = L // P          # 8 key/query tiles per half
KC = D // P          # 8 contraction chunks
DT4 = DG // P        # 4 head-pair chunks (128 ch each)

REPEAT = 1  # loop whole computation inside the NEFF (timing experiments only)
PHASES = "all"  # "all" | comma list of v,qk,attn,oproj + experiment flags

_CACHE = {}


def _on(name):
    return PHASES == "all" or name in PHASES.split(",")


def _x(name):
    """Experiment-only modifier: never active in the real kernel."""
    return PHASES != "all" and name in PHASES.split(",")


def _chunks512(a0, a1):
    """Split [a0, a1) at multiples of 512 (PSUM bank boundaries)."""
    out = []
    while a0 < a1:
        b1 = min(a1, (a0 // 512 + 1) * 512)
        out.append((a0, b1))
        a0 = b1
    return out


def _build():
    import concourse.tile_utils as tile_utils

    tile_utils.max_sbuf_usage = 204 * 1024

    nc = bacc.Bacc("TRN2", target_bir_lowering=False, debug=False, num_devices=8)

    xT = nc.dram_tensor("xT", [D, T], F16, kind="ExternalInput").ap()
    wq = nc.dram_tensor("wq", [D, DG], F16, kind="ExternalInput").ap()
    wk = nc.dram_tensor("wk", [D, DG], F16, kind="ExternalInput").ap()
    wv = nc.dram_tensor("wv", [D, DG], F16, kind="ExternalInput").ap()
    wo = nc.dram_tensor("wo", [DG, D], F16, kind="ExternalInput").ap()
    bqs = nc.dram_tensor("bqs", [DG], F32, kind="ExternalInput").ap()
    bks = nc.dram_tensor("bks", [DG], F32, kind="ExternalInput").ap()
    msk = nc.dram_tensor("msk", [3, P, P], F16, kind="ExternalInput").ap()
    out = nc.dram_tensor("out", [T, D], F32, kind="ExternalOutput").ap()

    views = dict(
        x0_v=xT.rearrange("(kc p) t -> p kc t", p=P)[:, :, 0:L],
        x1_v=xT.rearrange("(kc p) t -> p kc t", p=P)[:, :, L:T],
        wq_v=wq.rearrange("(kc p) m -> p kc m", p=P),    # [128, 8, 512]
        wk_v=wk.rearrange("(kc p) m -> p kc m", p=P),
        wv_v=wv.rearrange("(kc p) m -> p kc m", p=P),
        wo_v=wo.rearrange("(cc p) n -> p cc n", p=P),    # [128, 4, 1024]
        bqs=bqs,
        bks=bks,
        msk=msk,
        out=out,
    )

    with tile.TileContext(nc) as tc:
        with tc.tile_pool(name="persist", bufs=1) as pers:
            st = dict(
                x0_sb=pers.tile([P, KC, L], F16, name="x0_sb"),
                x1_sb=pers.tile([P, KC, L], F16, name="x1_sb"),
                wq_sb=pers.tile([P, KC, DG], F16, name="wq_sb"),
                wk_sb=pers.tile([P, KC, DG], F16, name="wk_sb"),
                wv_sb=pers.tile([P, KC, DG], F16, name="wv_sb"),
                wo_sb=pers.tile([P, DT4, D], F16, name="wo_sb"),
                qT=[pers.tile([P, T], F16, name=f"qT{c}") for c in range(DT4)],
                kT=[pers.tile([P, T], F16, name=f"kT{c}") for c in range(DT4)],
                v_xt=pers.tile([P, NT, HG * (HD + 1)], F16, name="v_xt"),
                v_x0=pers.tile([P, NT, HG * (HD + 1)], F16, name="v_x0"),
                ctxT=pers.tile([P, DT4, T], F16, name="ctxT"),
                bq_sb=pers.tile([P, DT4], F32, name="bq_sb"),
                bk_sb=pers.tile([P, DT4], F32, name="bk_sb"),
                m_strict=pers.tile([P, P], F16, name="m_strict"),
                m_incl=pers.tile([P, P], F16, name="m_incl"),
                m_diag=pers.tile([P, P], F16, name="m_diag"),
                ones_t=pers.tile([P, HD], F16, name="ones_t"),
            )
            nc.vector.memset(st["ones_t"], 1.0)
            if PHASES != "all":  # benign init for phase-subset timing builds
                for tl in st["qT"] + st["kT"] + [st["ctxT"]]:
                    nc.vector.memset(tl, 0.001)
                for vt in (st["v_xt"], st["v_x0"]):
                    nc.vector.memset(vt, 1.0)
            for vt in (st["v_xt"], st["v_x0"]):
                ones_v = vt.rearrange("p t (h c) -> p (t h) c", c=HD + 1)[
                    :, :, HD : HD + 1
                ]
                nc.vector.memset(ones_v, 1.0)

            for _rep in range(REPEAT):
                _phases(nc, tc, st, views)

    nc.compile()
    return nc


def _phases(nc, tc, st, views):
    x0_sb, x1_sb = st["x0_sb"], st["x1_sb"]
    ctxT = st["ctxT"]

    # ---------------- input DMAs (first-needed first; 2 queues) -------------
    nc.sync.dma_start(st["bq_sb"], views["bqs"].rearrange("(c p) -> p c", p=P))
    nc.sync.dma_start(st["bk_sb"], views["bks"].rearrange("(c p) -> p c", p=P))
    nc.sync.dma_start(st["m_strict"], views["msk"][0])
    nc.sync.dma_start(st["m_incl"], views["msk"][1])
    nc.sync.dma_start(st["m_diag"], views["msk"][2])
    nc.scalar.dma_start(x0_sb[:, :, 0:512], views["x0_v"][:, :, 0:512])
    nc.sync.dma_start(st["wv_sb"], views["wv_v"])
    nc.scalar.dma_start(x0_sb[:, :, 512:L], views["x0_v"][:, :, 512:L])
    nc.scalar.dma_start(x1_sb, views["x1_v"])
    for c in range(DT4):
        sl = slice(P * c, P * (c + 1))
        nc.sync.dma_start(st["wq_sb"][:, :, sl], views["wq_v"][:, :, sl])
        nc.sync.dma_start(st["wk_sb"][:, :, sl], views["wk_v"][:, :, sl])
    nc.scalar.dma_start(st["wo_sb"], views["wo_v"])

    # One shared PSUM ring for every phase (4 banks) + ctx accumulators
    # (4 banks): no pool transitions between phases.
    with (
        tc.tile_pool(name="scr", bufs=4, space="PSUM") as scr,
        tc.tile_pool(name="cps", bufs=4, space="PSUM") as cps,
        tc.tile_pool(name="atp", bufs=8) as atp,
        tc.tile_pool(name="tmp", bufs=4) as tmp,
    ):
        # ---------------- V pass ----------------
        # v[t, ch] tiles; per-head ones column accumulates denominators
        for t2 in range(T // P if _on("v") else 0):
            x_sb = x0_sb if t2 < NT else x1_sb
            dst = st["v_xt"] if t2 < NT else st["v_x0"]
            row = t2 % NT
            toff = P * (t2 % NT)
            ps = scr.tile([P, DG], F32, tag="ps", name=f"vp{t2}")
            for kc in range(KC):
                nc.tensor.matmul(
                    ps,
                    x_sb[:, kc, toff : toff + P],
                    st["wv_sb"][:, kc, :],
                    start=(kc == 0),
                    stop=(kc == KC - 1),
                )
            # ACT is idle this early; keep DVE free for attention work
            nc.scalar.activation(
                dst[:, row].rearrange("p (h c) -> p h c", c=HD + 1)[:, :, :HD],
                ps.rearrange("p (h c) -> p h c", c=HD),
                Act.Copy,
            )

        for c in range(DT4):
            if _on("qk"):
                _qk(nc, st, c, scr)
            if _on("attn"):
                _attn(nc, st, c, scr, cps, atp, tmp)

        # ---------------- O-projection ----------------
        for tt in range(T // P if _on("oproj") else 0):
            ops = [
                scr.tile([P, 512], F32, tag="ps", name=f"op{tt}_{nk}")
                for nk in range(2)
            ]
            for cc in range(DT4):
                stat = ctxT[:, cc, P * tt : P * (tt + 1)]
                for nk in range(2):
                    nc.tensor.matmul(
                        ops[nk],
                        stat,
                        st["wo_sb"][:, cc, 512 * nk : 512 * (nk + 1)],
                        start=(cc == 0),
                        stop=(cc == DT4 - 1),
                    )
            for nk in range(2):
                osb = tmp.tile([P, 512], F32, tag="osb", name=f"osb{tt}_{nk}")
                nc.scalar.activation(osb, ops[nk], Act.Copy)
                nc.sync.dma_start(
                    views["out"][P * tt : P * (tt + 1), 512 * nk : 512 * (nk + 1)],
                    osb,
                )


def _qk(nc, st, c, scr):
    """Q and K projections for head-pair chunk c (128 channels).
    Weight-chunk stationary, 2 moving 512-slabs per load."""
    for w_sb, b_sb, dst in (
        (st["wq_sb"], st["bq_sb"], st["qT"][c]),
        (st["wk_sb"], st["bk_sb"], st["kT"][c]),
    ):
        for sh in range(2):  # T halves; x0/x1 tiles
            x_sb = st["x0_sb"] if sh == 0 else st["x1_sb"]
            ps = [
                scr.tile([P, 512], F32, tag="ps", name=f"pp{c}_{sh}_{i}")
                for i in range(2)
            ]
            for kc in range(KC):
                stat = w_sb[:, kc, P * c : P * (c + 1)]
                for i in range(2):
                    nc.tensor.matmul(
                        ps[i],
                        stat,
                        x_sb[:, kc, 512 * i : 512 * (i + 1)],
                        start=(kc == 0),
                        stop=(kc == KC - 1),
                    )
            for i in range(2):
                # bias-add on DVE: ACT is saturated with exp in the
                # overlapping attention segment (q scale folded into Wq)
                with nc.allow_low_precision(reason="fp16 qk"):
                    nc.vector.tensor_scalar_add(
                        dst[:, L * sh + 512 * i : L * sh + 512 * (i + 1)],
                        ps[i],
                        b_sb[:, c : c + 1],
                    )


def _attn(nc, st, c, scr, cps, atp, tmp):
    """Sparse BD3LM attention for head pair (2c, 2c+1).
    Even head on partitions 0-63 of qT[c]/kT[c], odd on 64-127; their K=64
    score matmuls issue adjacently -> concurrent PE row groups. ctx psum
    [65, 512] per q bank: row 64 = softmax denominator (v ones column)."""
    qTc, kTc, ctxT = st["qT"][c], st["kT"][c], st["ctxT"]
    he, ho = 2 * c, 2 * c + 1
    rows = (slice(0, HD), slice(HD, 2 * HD))
    vcol = (slice((HD + 1) * he, (HD + 1) * (he + 1)),
            slice((HD + 1) * ho, (HD + 1) * (ho + 1)))

    for half in range(2):
        # per-q-bank accumulators, normalized as soon as the bank completes
        ctx = [
            [
                cps.tile([HD + 1, 512], F32, tag="ctx", name=f"cx{c}{half}{e}{bk}")
                for bk in range(2)
            ]
            for e in range(2)
        ]
        mask = st["m_strict"] if half == 0 else st["m_incl"]
        for j in range(NT):
            ats = []
            for a0, a1 in _chunks512(P * j, L):
                n = a1 - a0
                pair = []
                for e in range(2):
                    sc = scr.tile(
                        [P, 512], F32, tag="ps", name=f"sc{c}{j}{half}{a0}{e}"
                    )[:, :n]
                    nc.tensor.matmul(
                        sc,
                        kTc[rows[e], L + P * j : L + P * (j + 1)],
                        qTc[rows[e], L * half + a0 : L * half + a1],
                        start=True,
                        stop=True,
                    )
                    pair.append(sc)
                if _x("attn_sc"):
                    continue
                atl = []
                for e in range(2):
                    at = atp.tile(
                        [P, 512], F16, tag="at", name=f"at{c}{j}{half}{a0}{e}"
                    )[:, :n]
                    nc.scalar.activation(at, pair[e], Act.Exp)
                    if a0 == P * j:
                        nc.vector.tensor_mul(at[:, :P], at[:, :P], mask)
                    atl.append(at)
                ats.append((a0, a1, atl))
            if _x("attn_sc") or _x("attn_scexp"):
                continue
            for a0, a1, atl in ats:
                bk = a0 // 512
                last = half == 1 and (
                    (bk == 0 and j == 3) or (bk == 1 and j == NT - 1)
                )
                for e in range(2):
                    nc.tensor.matmul(
                        ctx[e][bk][:, a0 - 512 * bk : a1 - 512 * bk],
                        st["v_x0"][:, j, vcol[e]],
                        atl[e],
                        start=(j == 0),
                        stop=last,
                    )
            if j == 3 or j == NT - 1:
                bk = 0 if j == 3 else 1
                if half == 0 and not _x("attn_nodiag"):
                    _diag(nc, st, c, bk, ctx, scr, atp, rows, vcol, qTc, kTc)
                if _x("attn_nonorm"):
                    continue
                for e in range(2):
                    _norm_bank(nc, st, ctxT, c, half, e, bk, ctx[e][bk], tmp, scr)


def _diag(nc, st, c, ib, ctx, scr, atp, rows, vcol, qTc, kTc):
    """xt-xt block-diagonal tiles i = 4*ib .. 4*ib+3, one exp per head."""
    scd = []
    for e in range(2):
        t = scr.tile([P, 512], F32, tag="ps", name=f"scd{c}{ib}{e}")
        for i4 in range(4):
            i = 4 * ib + i4
            nc.tensor.matmul(
                t[:, P * i4 : P * (i4 + 1)],
                kTc[rows[e], P * i : P * (i + 1)],
                qTc[rows[e], P * i : P * (i + 1)],
                start=True,
                stop=True,
            )
        scd.append(t)
    for e in range(2):
        atd = atp.tile([P, 512], F16, tag="at", name=f"atd{c}{ib}{e}")
        nc.scalar.activation(atd, scd[e], Act.Exp)
        nc.vector.tensor_mul(
            atd.rearrange("p (i q) -> p i q", q=P),
            atd.rearrange("p (i q) -> p i q", q=P),
            st["m_diag"][:, None, :].to_broadcast((P, 4, P)),
        )
        for i4 in range(4):
            i = 4 * ib + i4
            nc.tensor.matmul(
                ctx[e][ib][:, P * i4 : P * (i4 + 1)],
                st["v_xt"][:, i, vcol[e]],
                atd[:, P * i4 : P * (i4 + 1)],
                start=False,
                stop=(i4 == 3),
            )


def _norm_bank(nc, st, ctxT, c, half, e, bk, ctx_tile, tmp, scr):
    """ctxT rows = ctx[:64] * (1/denom) for one 512-wide q bank.
    denom = ctx row 64. Odd head (e=1) lands on partitions 64-127 of ctxT
    via an SBUF-to-SBUF DMA (engines cannot move data across partitions)."""
    span = slice(L * half + 512 * bk, L * half + 512 * (bk + 1))
    recip = tmp.tile([P, 512], F16, tag="rc", name=f"rc{c}{half}{e}{bk}")
    with nc.allow_low_precision(reason="fp16 recip"):
        nc.vector.reciprocal(recip[HD : HD + 1, :], ctx_tile[HD : HD + 1, :])
    bc = scr.tile([P, 512], F32, tag="ps", name=f"bc{c}{half}{e}{bk}")[:HD, :]
    nc.tensor.matmul(
        bc,
        st["ones_t"][HD : HD + 1, :],
        recip[HD : HD + 1, :],
        start=True,
        stop=True,
    )
    # DVE ops may read at most one PSUM operand: stage bc in SBUF
    rb = tmp.tile([HD, 512], F16, tag="rb", name=f"rb{c}{half}{e}{bk}")
    with nc.allow_low_precision(reason="fp16 ctx"):
        nc.vector.tensor_copy(rb, bc)
        if e == 0:
            nc.vector.tensor_mul(ctxT[:HD, c, span], ctx_tile[:HD, :], rb)
        else:
            cs = tmp.tile([HD, 512], F16, tag="cs", name=f"cs{c}{half}{bk}")
            nc.vector.tensor_mul(cs, ctx_tile[:HD, :], rb)
            nc.sync.dma_start(ctxT[HD : 2 * HD, c, span], cs)


def _masks():
    q = np.arange(P)[None, :] // BS
    k = np.arange(P)[:, None] // BS
    m = np.zeros((3, P, P), np.float16)
    m[0] = (q > k).astype(np.float16)    # strict (xt q vs x0 k, same tile)
    m[1] = (q >= k).astype(np.float16)   # incl (x0 q vs x0 k, same tile)
    m[2] = (q == k).astype(np.float16)   # diag (xt q vs xt k, same tile)
    return m


def _in_maps(x, Wq, bq, Wk, bk, Wv, Wo):
    masks = _masks()
    scale = HD ** -0.5
    in_maps = []
    for core in range(8):
        b, g = core // 2, core % 2
        cols = slice(DG * g, DG * (g + 1))
        in_maps.append(
            {
                "xT": np.ascontiguousarray(x[b].T).astype(np.float16),
                "wq": (np.ascontiguousarray(Wq[:, cols]) * np.float32(scale)).astype(
                    np.float16
                ),
                "wk": np.ascontiguousarray(Wk[:, cols]).astype(np.float16),
                "wv": np.ascontiguousarray(Wv[:, cols]).astype(np.float16),
                "wo": np.ascontiguousarray(Wo[cols, :]).astype(np.float16),
                "bqs": np.ascontiguousarray(bq[cols]).astype(np.float32)
                * np.float32(scale),
                "bks": np.ascontiguousarray(bk[cols]).astype(np.float32),
                "msk": masks,
            }
        )
    return in_maps


def kernel(x, Wq, bq, Wk, bk, Wv, bv, Wo, bo, block_size=4, **_):
    x = np.asarray(x, np.float32)
    Wq, bq = np.asarray(Wq, np.float32), np.asarray(bq, np.float32)
    Wk, bk = np.asarray(Wk, np.float32), np.asarray(bk, np.float32)
    Wv, bv = np.asarray(Wv, np.float32), np.asarray(bv, np.float32)
    Wo, bo = np.asarray(Wo, np.float32), np.asarray(bo, np.float32)

    if "nc" not in _CACHE:
        _CACHE["nc"] = _build()
    nc = _CACHE["nc"]

    in_maps = _in_maps(x, Wq, bq, Wk, bk, Wv, Wo)
    _CACHE["last_in_maps"] = in_maps
    last_err = None
    for _attempt in range(6):
        try:
            res = run_bass_kernel_spmd(nc, in_maps, core_ids=list(range(8)), trace=False)
            break
        except Exception as e:  # transient NRT device flakes
            last_err = e
            msg = str(e)
            if "UNRECOVERABLE" not in msg and "UNAVAILABLE" not in msg:
                raise
            import time as _time

            import jax as _jax

            _time.sleep(5 * (_attempt + 1))
            try:
                _jax.clear_backends()
            except Exception:
                pass
    else:
        raise last_err

    corr = (bv @ Wo + bo).astype(np.float32)  # softmax rows sum to 1
    out = np.empty((B, T, D), np.float32)
    for b in range(B):
        out[b] = res.results[2 * b]["out"] + res.results[2 * b + 1]["out"] + corr
    return out


if __name__ == "__main__":
    rng = np.random.default_rng(0)
    inputs = {
        "x": rng.standard_normal((B, T, D)).astype(np.float32),
        "Wq": (rng.standard_normal((D, D)) / 32).astype(np.float32),
        "bq": np.zeros(D, np.float32),
        "Wk": (rng.standard_normal((D, D)) / 32).astype(np.float32),
        "bk": np.zeros(D, np.float32),
        "Wv": (rng.standard_normal((D, D)) / 32).astype(np.float32),
        "bv": np.zeros(D, np.float32),
        "Wo": (rng.standard_normal((D, D)) / 32).astype(np.float32),
        "bo": np.zeros(D, np.float32),
    }
    o = kernel(**inputs)
    print("ran", o.shape, o.dtype, float(np.abs(o).max()))


# revision 34
# speedup vs baseline: 1.2090x; 1.2090x over previous
"""BD3LM block-diffusion decoder layer on 8 trn2 NeuronCores.

Sharding: core = 2*b + g  (b = batch 0..3, g = head-group 0..1, 8 heads each).
Each core: QKV projections for its batch/head-group, sparse BD3LM attention
(only ~80 of 256 score tiles per head), O-projection against its Wo row-slice.
Host: sums the two group partials per batch and adds the (bv @ Wo + bo)
correction (softmax rows sum to 1, so the v-bias contributes exactly bv @ Wo).

All matmuls fp16 (1 cyc/row at every moving size vs f32r's 4 cyc/row under
256; fast weight load; half DMA; 2x DVE). fp32 PSUM accumulation throughout.

Every phase draws PSUM from ONE shared 4-slot [128,512] ring (+4 banks of
attention ctx accumulators) opened once per iteration - no pool transitions
between phases, so phase tails overlap the next phase's matmuls:
  V pass     v[t,ch] tiles with a per-head ones column (denominators free)
  for c in 0..3:
      QK(c)    weight-chunk stationary, 2 moving 512-slabs per load
      ATTN(c)  heads (2c,2c+1) on partitions 0-63/64-127: paired K=64 score
               matmuls issue adjacently -> concurrent PE row groups. Scores
               transposed [k,q]; exp on ACT (no max-subtraction; scores
               ~N(0,1)); per-q-bank normalize as soon as the bank completes.
  O-proj     ctxT chunk stationary, reused for both 512-col Wo halves
"""

import numpy as np

import concourse.bass as bass
import concourse.mybir as mybir
import concourse.tile as tile
from concourse import bacc
from concourse.bass_utils import run_bass_kernel_spmd

F32 = mybir.dt.float32
F16 = mybir.dt.float16
Act = mybir.ActivationFunctionType

B, T, D = 4, 2048, 1024
H, HD = 16, 64
L = T // 2           # 1024, length of each of [xt | x0]
BS = 4               # block size
G = 2                # head groups (cores per batch)
DG = D // G          # 512 channels per group
HG = H // G          # 8 heads per core
P = 128
NT = L // P          # 8 key/query tiles per half
KC = D // P          # 8 contraction chunks
DT4 = DG // P        # 4 head-pair chunks (128 ch each)

REPEAT = 1  # loop whole computation inside the NEFF (timing experiments only)
PHASES = "all"  # "all" | comma list of v,qk,attn,oproj + experiment flags

_CACHE = {}


def _on(name):
    return PHASES == "all" or name in PHASES.split(",")


def _x(name):
    """Experiment-only modifier: never active in the real kernel."""
    return PHASES != "all" and name in PHASES.split(",")


def _chunks512(a0, a1):
    """Split [a0, a1) at multiples of 512 (PSUM bank boundaries)."""
    out = []
    while a0 < a1:
        b1 = min(a1, (a0 // 512 + 1) * 512)
        out.append((a0, b1))
        a0 = b1
    return out


def _build():
    import concourse.tile_utils as tile_utils

    tile_utils.max_sbuf_usage = 204 * 1024

    nc = bacc.Bacc("TRN2", target_bir_lowering=False, debug=False, num_devices=8)

    xT = nc.dram_tensor("xT", [D, T], F16, kind="ExternalInput").ap()
    wq = nc.dram_tensor("wq", [D, DG], F16, kind="ExternalInput").ap()
    wk = nc.dram_tensor("wk", [D, DG], F16, kind="ExternalInput").ap()
    wv = nc.dram_tensor("wv", [D, DG], F16, kind="ExternalInput").ap()
    wo = nc.dram_tensor("wo", [DG, D], F16, kind="ExternalInput").ap()
    bqs = nc.dram_tensor("bqs", [DG], F32, kind="ExternalInput").ap()
    bks = nc.dram_tensor("bks", [DG], F32, kind="ExternalInput").ap()
    msk = nc.dram_tensor("msk", [3, P, P], F16, kind="ExternalInput").ap()
    out = nc.dram_tensor("out", [T, D], F32, kind="ExternalOutput").ap()

    views = dict(
        x0_v=xT.rearrange("(kc p) t -> p kc t", p=P)[:, :, 0:L],
        x1_v=xT.rearrange("(kc p) t -> p kc t", p=P)[:, :, L:T],
        wq_v=wq.rearrange("(kc p) m -> p kc m", p=P),    # [128, 8, 512]
        wk_v=wk.rearrange("(kc p) m -> p kc m", p=P),
        wv_v=wv.rearrange("(kc p) m -> p kc m", p=P),
        wo_v=wo.rearrange("(cc p) n -> p cc n", p=P),    # [128, 4, 1024]
        bqs=bqs,
        bks=bks,
        msk=msk,
        out=out,
    )

    with tile.TileContext(nc) as tc:
        with tc.tile_pool(name="persist", bufs=1) as pers:
            st = dict(
                x0_sb=pers.tile([P, KC, L], F16, name="x0_sb"),
                x1_sb=pers.tile([P, KC, L], F16, name="x1_sb"),
                wq_sb=pers.tile([P, KC, DG], F16, name="wq_sb"),
                wk_sb=pers.tile([P, KC, DG], F16, name="wk_sb"),
                wv_sb=pers.tile([P, KC, DG], F16, name="wv_sb"),
                wo_sb=pers.tile([P, DT4, D], F16, name="wo_sb"),
                qT=[pers.tile([P, T], F16, name=f"qT{c}") for c in range(DT4)],
                kT=[pers.tile([P, T], F16, name=f"kT{c}") for c in range(DT4)],
                v_xt=pers.tile([P, NT, HG * (HD + 1)], F16, name="v_xt"),
                v_x0=pers.tile([P, NT, HG * (HD + 1)], F16, name="v_x0"),
                ctxT=pers.tile([P, DT4, T], F16, name="ctxT"),
                bq_sb=pers.tile([P, DT4], F32, name="bq_sb"),
                bk_sb=pers.tile([P, DT4], F32, name="bk_sb"),
                m_strict=pers.tile([P, P], F16, name="m_strict"),
                m_incl=pers.tile([P, P], F16, name="m_incl"),
                m_diag=pers.tile([P, P], F16, name="m_diag"),
                ones_t=pers.tile([P, HD], F16, name="ones_t"),
            )
            nc.vector.memset(st["ones_t"], 1.0)
            if PHASES != "all":  # benign init for phase-subset timing builds
                for tl in st["qT"] + st["kT"] + [st["ctxT"]]:
                    nc.vector.memset(tl, 0.001)
                for vt in (st["v_xt"], st["v_x0"]):
                    nc.vector.memset(vt, 1.0)
            for vt in (st["v_xt"], st["v_x0"]):
                ones_v = vt.rearrange("p t (h c) -> p (t h) c", c=HD + 1)[
                    :, :, HD : HD + 1
                ]
                nc.vector.memset(ones_v, 1.0)

            for _rep in range(REPEAT):
                _phases(nc, tc, st, views)

    nc.compile()
    return nc


def _phases(nc, tc, st, views):
    x0_sb, x1_sb = st["x0_sb"], st["x1_sb"]
    ctxT = st["ctxT"]

    # ---------------- input DMAs (first-needed first; 2 queues) -------------
    nc.sync.dma_start(st["bq_sb"], views["bqs"].rearrange("(c p) -> p c", p=P))
    nc.sync.dma_start(st["bk_sb"], views["bks"].rearrange("(c p) -> p c", p=P))
    nc.sync.dma_start(st["m_strict"], views["msk"][0])
    nc.sync.dma_start(st["m_incl"], views["msk"][1])
    nc.sync.dma_start(st["m_diag"], views["msk"][2])
    nc.scalar.dma_start(x0_sb[:, :, 0:512], views["x0_v"][:, :, 0:512])
    nc.sync.dma_start(st["wv_sb"], views["wv_v"])
    nc.scalar.dma_start(x0_sb[:, :, 512:L], views["x0_v"][:, :, 512:L])
    nc.scalar.dma_start(x1_sb, views["x1_v"])
    for c in range(DT4):
        sl = slice(P * c, P * (c + 1))
        nc.sync.dma_start(st["wq_sb"][:, :, sl], views["wq_v"][:, :, sl])
        nc.sync.dma_start(st["wk_sb"][:, :, sl], views["wk_v"][:, :, sl])
    nc.scalar.dma_start(st["wo_sb"], views["wo_v"])

    # One shared PSUM ring for every phase (4 banks) + ctx accumulators
    # (4 banks): no pool transitions between phases.
    with (
        tc.tile_pool(name="scr", bufs=4, space="PSUM") as scr,
        tc.tile_pool(name="cps", bufs=4, space="PSUM") as cps,
        tc.tile_pool(name="atp", bufs=12) as atp,
        tc.tile_pool(name="tmp", bufs=4) as tmp,
    ):
        # ---------------- V pass ----------------
        # v[t, ch] tiles; per-head ones column accumulates denominators
        for t2 in range(T // P if _on("v") else 0):
            x_sb = x0_sb if t2 < NT else x1_sb
            dst = st["v_xt"] if t2 < NT else st["v_x0"]
            row = t2 % NT
            toff = P * (t2 % NT)
            ps = scr.tile([P, DG], F32, tag="ps", name=f"vp{t2}")
            for kc in range(KC):
                nc.tensor.matmul(
                    ps,
                    x_sb[:, kc, toff : toff + P],
                    st["wv_sb"][:, kc, :],
                    start=(kc == 0),
                    stop=(kc == KC - 1),
                )
            # ACT is idle this early; keep DVE free for attention work
            nc.scalar.activation(
                dst[:, row].rearrange("p (h c) -> p h c", c=HD + 1)[:, :, :HD],
                ps.rearrange("p (h c) -> p h c", c=HD),
                Act.Copy,
            )

        for c in range(DT4):
            if _on("qk"):
                _qk(nc, st, c, scr)
            if _on("attn"):
                _attn(nc, st, c, scr, cps, atp, tmp)

        # ---------------- O-projection ----------------
        for tt in range(T // P if _on("oproj") else 0):
            ops = [
                scr.tile([P, 512], F32, tag="ps", name=f"op{tt}_{nk}")
                for nk in range(2)
            ]
            for cc in range(DT4):
                stat = ctxT[:, cc, P * tt : P * (tt + 1)]
                for nk in range(2):
                    nc.tensor.matmul(
                        ops[nk],
                        stat,
                        st["wo_sb"][:, cc, 512 * nk : 512 * (nk + 1)],
                        start=(cc == 0),
                        stop=(cc == DT4 - 1),
                    )
            for nk in range(2):
                osb = tmp.tile([P, 512], F32, tag="osb", name=f"osb{tt}_{nk}")
                nc.scalar.activation(osb, ops[nk], Act.Copy)
                nc.sync.dma_start(
                    views["out"][P * tt : P * (tt + 1), 512 * nk : 512 * (nk + 1)],
                    osb,
                )


def _qk(nc, st, c, scr):
    """Q and K projections for head-pair chunk c (128 channels).
    Weight-chunk stationary, 2 moving 512-slabs per load."""
    for w_sb, b_sb, dst in (
        (st["wq_sb"], st["bq_sb"], st["qT"][c]),
        (st["wk_sb"], st["bk_sb"], st["kT"][c]),
    ):
        for sh in range(2):  # T halves; x0/x1 tiles
            x_sb = st["x0_sb"] if sh == 0 else st["x1_sb"]
            ps = [
                scr.tile([P, 512], F32, tag="ps", name=f"pp{c}_{sh}_{i}")
                for i in range(2)
            ]
            for kc in range(KC):
                stat = w_sb[:, kc, P * c : P * (c + 1)]
                for i in range(2):
                    nc.tensor.matmul(
                        ps[i],
                        stat,
                        x_sb[:, kc, 512 * i : 512 * (i + 1)],
                        start=(kc == 0),
                        stop=(kc == KC - 1),
                    )
            for i in range(2):
                # bias-add on DVE: ACT is saturated with exp in the
                # overlapping attention segment (q scale folded into Wq)
                with nc.allow_low_precision(reason="fp16 qk"):
                    nc.vector.tensor_scalar_add(
                        dst[:, L * sh + 512 * i : L * sh + 512 * (i + 1)],
                        ps[i],
                        b_sb[:, c : c + 1],
                    )


def _attn(nc, st, c, scr, cps, atp, tmp):
    """Sparse BD3LM attention for head pair (2c, 2c+1).
    Even head on partitions 0-63 of qT[c]/kT[c], odd on 64-127; their K=64
    score matmuls issue adjacently -> concurrent PE row groups. ctx psum
    [65, 512] per q bank: row 64 = softmax denominator (v ones column)."""
    qTc, kTc, ctxT = st["qT"][c], st["kT"][c], st["ctxT"]
    he, ho = 2 * c, 2 * c + 1
    rows = (slice(0, HD), slice(HD, 2 * HD))
    vcol = (slice((HD + 1) * he, (HD + 1) * (he + 1)),
            slice((HD + 1) * ho, (HD + 1) * (ho + 1)))

    masks = (st["m_strict"], st["m_incl"])
    # q-bank-major, both halves interleaved round-robin over j: the two
    # halves share each kv_j stationary, and one half's exp tail fills the
    # other's PE slots. ctx accumulators: 4 live tiles (e,o) x (half0,1).
    for qb in range(2):
        ctx = [
            [
                cps.tile([HD + 1, 512], F32, tag="ctx", name=f"cx{c}{qb}{h}{e}")
                for e in range(2)
            ]
            for h in range(2)
        ]
        jmax = 4 if qb == 0 else NT
        for j in range(jmax):
            off = max(0, P * j - 512 * qb)  # start col within this q bank
            for half in range(2):
                q0 = L * half + 512 * qb
                pair = []
                for e in range(2):
                    sc = scr.tile(
                        [P, 512], F32, tag="ps", name=f"sc{c}{qb}{j}{half}{e}"
                    )[:, off:]
                    nc.tensor.matmul(
                        sc,
                        kTc[rows[e], L + P * j : L + P * (j + 1)],
                        qTc[rows[e], q0 + off : q0 + 512],
                        start=True,
                        stop=True,
                    )
                    pair.append(sc)
                if _x("attn_sc"):
                    continue
                atl = []
                for e in range(2):
                    at = atp.tile(
                        [P, 512], F16, tag="at", name=f"at{c}{qb}{j}{half}{e}"
                    )[:, off:]
                    nc.scalar.activation(at, pair[e], Act.Exp)
                    if 4 * qb <= j < 4 * (qb + 1):
                        moff = P * j - 512 * qb - off  # == 0
                        nc.vector.tensor_mul(
                            at[:, moff : moff + P],
                            at[:, moff : moff + P],
                            masks[half],
                        )
                    atl.append(at)
                if _x("attn_scexp"):
                    continue
                last = half == 1 and j == jmax - 1
                for e in range(2):
                    nc.tensor.matmul(
                        ctx[half][e][:, off:],
                        st["v_x0"][:, j, vcol[e]],
                        atl[e],
                        start=(j == 0),
                        stop=last,
                    )
        if _x("attn_sc") or _x("attn_scexp"):
            continue
        if not _x("attn_nodiag"):
            _diag(nc, st, c, qb, ctx[0], scr, atp, rows, vcol, qTc, kTc)
        if _x("attn_nonorm"):
            continue
        for half in range(2):
            for e in range(2):
                _norm_bank(
                    nc, st, ctxT, c, half, e, qb, ctx[half][e], tmp, scr
                )


def _diag(nc, st, c, ib, ctx, scr, atp, rows, vcol, qTc, kTc):
    """xt-xt block-diagonal tiles i = 4*ib .. 4*ib+3, one exp per head."""
    scd = []
    for e in range(2):
        t = scr.tile([P, 512], F32, tag="ps", name=f"scd{c}{ib}{e}")
        for i4 in range(4):
            i = 4 * ib + i4
            nc.tensor.matmul(
                t[:, P * i4 : P * (i4 + 1)],
                kTc[rows[e], P * i : P * (i + 1)],
                qTc[rows[e], P * i : P * (i + 1)],
                start=True,
                stop=True,
            )
        scd.append(t)
    for e in range(2):
        atd = atp.tile([P, 512], F16, tag="at", name=f"atd{c}{ib}{e}")
        nc.scalar.activation(atd, scd[e], Act.Exp)
        nc.vector.tensor_mul(
            atd.rearrange("p (i q) -> p i q", q=P),
            atd.rearrange("p (i q) -> p i q", q=P),
            st["m_diag"][:, None, :].to_broadcast((P, 4, P)),
        )
        for i4 in range(4):
            i = 4 * ib + i4
            nc.tensor.matmul(
                ctx[e][:, P * i4 : P * (i4 + 1)],
                st["v_xt"][:, i, vcol[e]],
                atd[:, P * i4 : P * (i4 + 1)],
                start=False,
                stop=(i4 == 3),
            )


def _norm_bank(nc, st, ctxT, c, half, e, bk, ctx_tile, tmp, scr):
    """ctxT rows = ctx[:64] * (1/denom) for one 512-wide q bank.
    denom = ctx row 64. Odd head (e=1) lands on partitions 64-127 of ctxT
    via an SBUF-to-SBUF DMA (engines cannot move data across partitions)."""
    span = slice(L * half + 512 * bk, L * half + 512 * (bk + 1))
    recip = tmp.tile([P, 512], F16, tag="rc", name=f"rc{c}{half}{e}{bk}")
    with nc.allow_low_precision(reason="fp16 recip"):
        nc.vector.reciprocal(recip[HD : HD + 1, :], ctx_tile[HD : HD + 1, :])
    bc = scr.tile([P, 512], F32, tag="ps", name=f"bc{c}{half}{e}{bk}")[:HD, :]
    nc.tensor.matmul(
        bc,
        st["ones_t"][HD : HD + 1, :],
        recip[HD : HD + 1, :],
        start=True,
        stop=True,
    )
    # DVE ops may read at most one PSUM operand: stage bc in SBUF
    rb = tmp.tile([HD, 512], F16, tag="rb", name=f"rb{c}{half}{e}{bk}")
    with nc.allow_low_precision(reason="fp16 ctx"):
        nc.vector.tensor_copy(rb, bc)
        if e == 0:
            nc.vector.tensor_mul(ctxT[:HD, c, span], ctx_tile[:HD, :], rb)
        else:
            cs = tmp.tile([HD, 512], F16, tag="cs", name=f"cs{c}{half}{bk}")
            nc.vector.tensor_mul(cs, ctx_tile[:HD, :], rb)
            nc.sync.dma_start(ctxT[HD : 2 * HD, c, span], cs)


def _masks():
    q = np.arange(P)[None, :] // BS
    k = np.arange(P)[:, None] // BS
    m = np.zeros((3, P, P), np.float16)
    m[0] = (q > k).astype(np.float16)    # strict (xt q vs x0 k, same tile)
    m[1] = (q >= k).astype(np.float16)   # incl (x0 q vs x0 k, same tile)
    m[2] = (q == k).astype(np.float16)   # diag (xt q vs xt k, same tile)
    return m


def _in_maps(x, Wq, bq, Wk, bk, Wv, Wo):
    masks = _masks()
    scale = HD ** -0.5
    in_maps = []
    for core in range(8):
        b, g = core // 2, core % 2
        cols = slice(DG * g, DG * (g + 1))
        in_maps.append(
            {
                "xT": np.ascontiguousarray(x[b].T).astype(np.float16),
                "wq": (np.ascontiguousarray(Wq[:, cols]) * np.float32(scale)).astype(
                    np.float16
                ),
                "wk": np.ascontiguousarray(Wk[:, cols]).astype(np.float16),
                "wv": np.ascontiguousarray(Wv[:, cols]).astype(np.float16),
                "wo": np.ascontiguousarray(Wo[cols, :]).astype(np.float16),
                "bqs": np.ascontiguousarray(bq[cols]).astype(np.float32)
                * np.float32(scale),
                "bks": np.ascontiguousarray(bk[cols]).astype(np.float32),
                "msk": masks,
            }
        )
    return in_maps


def kernel(x, Wq, bq, Wk, bk, Wv, bv, Wo, bo, block_size=4, **_):
    x = np.asarray(x, np.float32)
    Wq, bq = np.asarray(Wq, np.float32), np.asarray(bq, np.float32)
    Wk, bk = np.asarray(Wk, np.float32), np.asarray(bk, np.float32)
    Wv, bv = np.asarray(Wv, np.float32), np.asarray(bv, np.float32)
    Wo, bo = np.asarray(Wo, np.float32), np.asarray(bo, np.float32)

    if "nc" not in _CACHE:
        _CACHE["nc"] = _build()
    nc = _CACHE["nc"]

    in_maps = _in_maps(x, Wq, bq, Wk, bk, Wv, Wo)
    _CACHE["last_in_maps"] = in_maps
    last_err = None
    for _attempt in range(6):
        try:
            res = run_bass_kernel_spmd(nc, in_maps, core_ids=list(range(8)), trace=False)
            break
        except Exception as e:  # transient NRT device flakes
            last_err = e
            msg = str(e)
            if "UNRECOVERABLE" not in msg and "UNAVAILABLE" not in msg:
                raise
            import time as _time

            import jax as _jax

            _time.sleep(5 * (_attempt + 1))
            try:
                _jax.clear_backends()
            except Exception:
                pass
    else:
        raise last_err

    corr = (bv @ Wo + bo).astype(np.float32)  # softmax rows sum to 1
    out = np.empty((B, T, D), np.float32)
    for b in range(B):
        out[b] = res.results[2 * b]["out"] + res.results[2 * b + 1]["out"] + corr
    return out


if __name__ == "__main__":
    rng = np.random.default_rng(0)
    inputs = {
        "x": rng.standard_normal((B, T, D)).astype(np.float32),
        "Wq": (rng.standard_normal((D, D)) / 32).astype(np.float32),
        "bq": np.zeros(D, np.float32),
        "Wk": (rng.standard_normal((D, D)) / 32).astype(np.float32),
        "bk": np.zeros(D, np.float32),
        "Wv": (rng.standard_normal((D, D)) / 32).astype(np.float32),
        "bv": np.zeros(D, np.float32),
        "Wo": (rng.standard_normal((D, D)) / 32).astype(np.float32),
        "bo": np.zeros(D, np.float32),
    }
    o = kernel(**inputs)
    print("ran", o.shape, o.dtype, float(np.abs(o).max()))


# revision 43
# speedup vs baseline: 2.0277x; 1.6772x over previous
"""BD3LM block-diffusion decoder layer on 8 trn2 NeuronCores.

Sharding: core = 2*b + g  (b = batch 0..3, g = head-group 0..1, 8 heads each).
Each core: QKV projections for its batch/head-group, sparse BD3LM attention
(only ~80 of 256 score tiles per head), O-projection against its Wo row-slice.
Host: sums the two group partials per batch and adds the (bv @ Wo + bo)
correction (softmax rows sum to 1, so the v-bias contributes exactly bv @ Wo).

All matmuls fp16 (1 cyc/row at every moving size vs f32r's 4 cyc/row under
256; fast weight load; half DMA; 2x DVE). fp32 PSUM accumulation throughout.

Every phase draws PSUM from ONE shared 4-slot [128,512] ring (+4 banks of
attention ctx accumulators) opened once per iteration - no pool transitions
between phases, so phase tails overlap the next phase's matmuls:
  V pass     v[t,ch] tiles with a per-head ones column (denominators free)
  for c in 0..3:
      QK(c)    weight-chunk stationary, 2 moving 512-slabs per load
      ATTN(c)  heads (2c,2c+1) on partitions 0-63/64-127: paired K=64 score
               matmuls issue adjacently -> concurrent PE row groups. Scores
               transposed [k,q]; exp on ACT (no max-subtraction; scores
               ~N(0,1)); per-q-bank normalize as soon as the bank completes.
  O-proj     ctxT chunk stationary, reused for both 512-col Wo halves
"""

import numpy as np

import concourse.bass as bass
import concourse.mybir as mybir
import concourse.tile as tile
from concourse import bacc
from concourse.bass_utils import run_bass_kernel_spmd

F32 = mybir.dt.float32
F16 = mybir.dt.float16
Act = mybir.ActivationFunctionType

B, T, D = 4, 2048, 1024
H, HD = 16, 64
L = T // 2           # 1024, length of each of [xt | x0]
BS = 4               # block size
G = 2                # head groups (cores per batch)
DG = D // G          # 512 channels per group
HG = H // G          # 8 heads per core
P = 128
NT = L // P          # 8 key/query tiles per half
KC = D // P          # 8 contraction chunks
DT4 = DG // P        # 4 head-pair chunks (128 ch each)

REPEAT = 1  # loop whole computation inside the NEFF (timing experiments only)
PHASES = "all"  # "all" | comma list of v,qk,attn,oproj + experiment flags

_CACHE = {}


def _on(name):
    return PHASES == "all" or name in PHASES.split(",")


def _x(name):
    """Experiment-only modifier: never active in the real kernel."""
    return PHASES != "all" and name in PHASES.split(",")


def _chunks512(a0, a1):
    """Split [a0, a1) at multiples of 512 (PSUM bank boundaries)."""
    out = []
    while a0 < a1:
        b1 = min(a1, (a0 // 512 + 1) * 512)
        out.append((a0, b1))
        a0 = b1
    return out


def _build():
    import concourse.tile_utils as tile_utils

    tile_utils.max_sbuf_usage = 204 * 1024

    nc = bacc.Bacc("TRN2", target_bir_lowering=False, debug=False, num_devices=8)

    xT = nc.dram_tensor("xT", [D, T], F16, kind="ExternalInput").ap()
    wq = nc.dram_tensor("wq", [D, DG], F16, kind="ExternalInput").ap()
    wk = nc.dram_tensor("wk", [D, DG], F16, kind="ExternalInput").ap()
    wv = nc.dram_tensor("wv", [D, DG], F16, kind="ExternalInput").ap()
    wo = nc.dram_tensor("wo", [DG, D], F16, kind="ExternalInput").ap()
    bqs = nc.dram_tensor("bqs", [DG], F32, kind="ExternalInput").ap()
    bks = nc.dram_tensor("bks", [DG], F32, kind="ExternalInput").ap()
    msk = nc.dram_tensor("msk", [3, P, P], F16, kind="ExternalInput").ap()
    out = nc.dram_tensor("out", [T, D], F32, kind="ExternalOutput").ap()

    views = dict(
        x0_v=xT.rearrange("(kc p) t -> p kc t", p=P)[:, :, 0:L],
        x1_v=xT.rearrange("(kc p) t -> p kc t", p=P)[:, :, L:T],
        wq_v=wq.rearrange("(kc p) m -> p kc m", p=P),    # [128, 8, 512]
        wk_v=wk.rearrange("(kc p) m -> p kc m", p=P),
        wv_v=wv.rearrange("(kc p) m -> p kc m", p=P),
        wo_v=wo.rearrange("(cc p) n -> p cc n", p=P),    # [128, 4, 1024]
        bqs=bqs,
        bks=bks,
        msk=msk,
        out=out,
    )

    with tile.TileContext(nc) as tc:
        with tc.tile_pool(name="persist", bufs=1) as pers:
            st = dict(
                x0_sb=pers.tile([P, KC, L], F16, name="x0_sb"),
                x1_sb=pers.tile([P, KC, L], F16, name="x1_sb"),
                wq_sb=pers.tile([P, KC, DG], F16, name="wq_sb"),
                wk_sb=pers.tile([P, KC, DG], F16, name="wk_sb"),
                wv_sb=pers.tile([P, KC, DG], F16, name="wv_sb"),
                wo_sb=pers.tile([P, DT4, D], F16, name="wo_sb"),
                qT=[pers.tile([P, T], F16, name=f"qT{c}") for c in range(DT4)],
                kT=[pers.tile([P, T], F16, name=f"kT{c}") for c in range(DT4)],
                v_xt=pers.tile([P, NT, HG * (HD + 1)], F16, name="v_xt"),
                v_x0=pers.tile([P, NT, HG * (HD + 1)], F16, name="v_x0"),
                ctxT=pers.tile([P, DT4, T], F16, name="ctxT"),
                bq_sb=pers.tile([P, DT4], F32, name="bq_sb"),
                bk_sb=pers.tile([P, DT4], F32, name="bk_sb"),
                m_strict=pers.tile([P, P], F16, name="m_strict"),
                m_incl=pers.tile([P, P], F16, name="m_incl"),
                m_diag=pers.tile([P, P], F16, name="m_diag"),
                ones_t=pers.tile([P, HD], F16, name="ones_t"),
            )
            nc.vector.memset(st["ones_t"], 1.0)
            if PHASES != "all":  # benign init for phase-subset timing builds
                for tl in st["qT"] + st["kT"] + [st["ctxT"]]:
                    nc.vector.memset(tl, 0.001)
                for vt in (st["v_xt"], st["v_x0"]):
                    nc.vector.memset(vt, 1.0)
            for vt in (st["v_xt"], st["v_x0"]):
                ones_v = vt.rearrange("p t (h c) -> p (t h) c", c=HD + 1)[
                    :, :, HD : HD + 1
                ]
                nc.vector.memset(ones_v, 1.0)

            for _rep in range(REPEAT):
                _phases(nc, tc, st, views)

    nc.compile()
    return nc


def _phases(nc, tc, st, views):
    x0_sb, x1_sb = st["x0_sb"], st["x1_sb"]
    ctxT = st["ctxT"]

    # ---------------- input DMAs (first-needed first; 2 queues) -------------
    nc.sync.dma_start(st["bq_sb"], views["bqs"].rearrange("(c p) -> p c", p=P))
    nc.sync.dma_start(st["bk_sb"], views["bks"].rearrange("(c p) -> p c", p=P))
    nc.sync.dma_start(st["m_strict"], views["msk"][0])
    nc.sync.dma_start(st["m_incl"], views["msk"][1])
    nc.sync.dma_start(st["m_diag"], views["msk"][2])
    nc.scalar.dma_start(x0_sb[:, :, 0:512], views["x0_v"][:, :, 0:512])
    nc.sync.dma_start(st["wv_sb"], views["wv_v"])
    nc.scalar.dma_start(x0_sb[:, :, 512:L], views["x0_v"][:, :, 512:L])
    nc.scalar.dma_start(x1_sb, views["x1_v"])
    for c in range(DT4):
        sl = slice(P * c, P * (c + 1))
        nc.sync.dma_start(st["wq_sb"][:, :, sl], views["wq_v"][:, :, sl])
        nc.sync.dma_start(st["wk_sb"][:, :, sl], views["wk_v"][:, :, sl])
    nc.scalar.dma_start(st["wo_sb"], views["wo_v"])

    # One shared PSUM ring for every phase (4 banks) + ctx accumulators
    # (4 banks): no pool transitions between phases.
    with (
        tc.tile_pool(name="scr", bufs=4, space="PSUM") as scr,
        tc.tile_pool(name="cps", bufs=4, space="PSUM") as cps,
        tc.tile_pool(name="atp", bufs=12) as atp,
        tc.tile_pool(name="tmp", bufs=4) as tmp,
    ):
        # ---------------- V pass ----------------
        # v[t, ch] tiles; per-head ones column accumulates denominators
        for t2 in range(T // P if _on("v") else 0):
            x_sb = x0_sb if t2 < NT else x1_sb
            dst = st["v_xt"] if t2 < NT else st["v_x0"]
            row = t2 % NT
            toff = P * (t2 % NT)
            ps = scr.tile([P, DG], F32, tag="ps", name=f"vp{t2}")
            for kc in range(KC):
                nc.tensor.matmul(
                    ps,
                    x_sb[:, kc, toff : toff + P],
                    st["wv_sb"][:, kc, :],
                    start=(kc == 0),
                    stop=(kc == KC - 1),
                )
            # ACT is idle this early; keep DVE free for attention work
            nc.scalar.activation(
                dst[:, row].rearrange("p (h c) -> p h c", c=HD + 1)[:, :, :HD],
                ps.rearrange("p (h c) -> p h c", c=HD),
                Act.Copy,
            )

        def _oproj(tts):
            for tt in tts:
                ops = [
                    scr.tile([P, 512], F32, tag="ps", name=f"op{tt}_{nk}")
                    for nk in range(2)
                ]
                for cc in range(DT4):
                    stat = ctxT[:, cc, P * tt : P * (tt + 1)]
                    for nk in range(2):
                        nc.tensor.matmul(
                            ops[nk],
                            stat,
                            st["wo_sb"][:, cc, 512 * nk : 512 * (nk + 1)],
                            start=(cc == 0),
                            stop=(cc == DT4 - 1),
                        )
                for nk in range(2):
                    osb = tmp.tile([P, 512], F32, tag="osb", name=f"osb{tt}_{nk}")
                    nc.scalar.activation(osb, ops[nk], Act.Copy)
                    nc.sync.dma_start(
                        views["out"][P * tt : P * (tt + 1), 512 * nk : 512 * (nk + 1)],
                        osb,
                    )

        # NOTE: dripping O-proj tiles into attn(3)'s tail (overlap) produced
        # wrong results on HW (race vs the odd-head ctxT DMA that CoreSim
        # does not catch) - keep O-proj strictly after attention.
        for c in range(DT4):
            if _on("qk"):
                _qk(nc, st, c, scr)
            if _on("attn"):
                _attn(nc, st, c, scr, cps, atp, tmp)
        if _on("oproj"):
            _oproj(list(range(T // P)))


def _qk(nc, st, c, scr):
    """Q and K projections for head-pair chunk c (128 channels).
    Weight-chunk stationary, 2 moving 512-slabs per load."""
    for w_sb, b_sb, dst in (
        (st["wq_sb"], st["bq_sb"], st["qT"][c]),
        (st["wk_sb"], st["bk_sb"], st["kT"][c]),
    ):
        for sh in range(2):  # T halves; x0/x1 tiles
            x_sb = st["x0_sb"] if sh == 0 else st["x1_sb"]
            ps = [
                scr.tile([P, 512], F32, tag="ps", name=f"pp{c}_{sh}_{i}")
                for i in range(2)
            ]
            for kc in range(KC):
                stat = w_sb[:, kc, P * c : P * (c + 1)]
                for i in range(2):
                    nc.tensor.matmul(
                        ps[i],
                        stat,
                        x_sb[:, kc, 512 * i : 512 * (i + 1)],
                        start=(kc == 0),
                        stop=(kc == KC - 1),
                    )
            for i in range(2):
                # bias-add on DVE: ACT is saturated with exp in the
                # overlapping attention segment (q scale folded into Wq)
                with nc.allow_low_precision(reason="fp16 qk"):
                    nc.vector.tensor_scalar_add(
                        dst[:, L * sh + 512 * i : L * sh + 512 * (i + 1)],
                        ps[i],
                        b_sb[:, c : c + 1],
                    )


def _attn(nc, st, c, scr, cps, atp, tmp, post_qb=None):
    """Sparse BD3LM attention for head pair (2c, 2c+1).
    Even head on partitions 0-63 of qT[c]/kT[c], odd on 64-127; their K=64
    score matmuls issue adjacently -> concurrent PE row groups. ctx psum
    [65, 512] per q bank: row 64 = softmax denominator (v ones column)."""
    qTc, kTc, ctxT = st["qT"][c], st["kT"][c], st["ctxT"]
    he, ho = 2 * c, 2 * c + 1
    rows = (slice(0, HD), slice(HD, 2 * HD))
    vcol = (slice((HD + 1) * he, (HD + 1) * (he + 1)),
            slice((HD + 1) * ho, (HD + 1) * (ho + 1)))

    masks = (st["m_strict"], st["m_incl"])
    # q-bank-major, both halves interleaved round-robin over j: the two
    # halves share each kv_j stationary, and one half's exp tail fills the
    # other's PE slots. ctx accumulators: 4 live tiles (e,o) x (half0,1).
    for qb in range(2):
        ctx = [
            [
                cps.tile([HD + 1, 512], F32, tag="ctx", name=f"cx{c}{qb}{h}{e}")
                for e in range(2)
            ]
            for h in range(2)
        ]
        jmax = 4 if qb == 0 else NT
        for j in range(jmax):
            off = max(0, P * j - 512 * qb)  # start col within this q bank
            for half in range(2):
                q0 = L * half + 512 * qb
                pair = []
                for e in range(2):
                    sc = scr.tile(
                        [P, 512], F32, tag="ps", name=f"sc{c}{qb}{j}{half}{e}"
                    )[:, off:]
                    nc.tensor.matmul(
                        sc,
                        kTc[rows[e], L + P * j : L + P * (j + 1)],
                        qTc[rows[e], q0 + off : q0 + 512],
                        start=True,
                        stop=True,
                    )
                    pair.append(sc)
                if _x("attn_sc"):
                    continue
                atl = []
                for e in range(2):
                    at = atp.tile(
                        [P, 512], F16, tag="at", name=f"at{c}{qb}{j}{half}{e}"
                    )[:, off:]
                    nc.scalar.activation(at, pair[e], Act.Exp)
                    if 4 * qb <= j < 4 * (qb + 1):
                        moff = P * j - 512 * qb - off  # == 0
                        nc.vector.tensor_mul(
                            at[:, moff : moff + P],
                            at[:, moff : moff + P],
                            masks[half],
                        )
                    atl.append(at)
                if _x("attn_scexp"):
                    continue
                last = half == 1 and j == jmax - 1
                for e in range(2):
                    nc.tensor.matmul(
                        ctx[half][e][:, off:],
                        st["v_x0"][:, j, vcol[e]],
                        atl[e],
                        start=(j == 0),
                        stop=last,
                    )
            if post_qb is not None and qb == 1 and j >= 1:
                # qb0's normalize has had >=1 j-round to drain: drip one
                # O-proj tile group into the PE queue per round
                post_qb(False)
        if _x("attn_sc") or _x("attn_scexp"):
            continue
        if not _x("attn_nodiag"):
            _diag(nc, st, c, qb, ctx[0], scr, atp, rows, vcol, qTc, kTc)
        if _x("attn_nonorm"):
            continue
        for half in range(2):
            for e in range(2):
                _norm_bank(
                    nc, st, ctxT, c, half, e, qb, ctx[half][e], tmp, scr
                )
        if post_qb is not None and qb == 1:
            post_qb(True)


def _diag(nc, st, c, ib, ctx, scr, atp, rows, vcol, qTc, kTc):
    """xt-xt block-diagonal tiles i = 4*ib .. 4*ib+3, one exp per head."""
    scd = []
    for e in range(2):
        t = scr.tile([P, 512], F32, tag="ps", name=f"scd{c}{ib}{e}")
        for i4 in range(4):
            i = 4 * ib + i4
            nc.tensor.matmul(
                t[:, P * i4 : P * (i4 + 1)],
                kTc[rows[e], P * i : P * (i + 1)],
                qTc[rows[e], P * i : P * (i + 1)],
                start=True,
                stop=True,
            )
        scd.append(t)
    for e in range(2):
        atd = atp.tile([P, 512], F16, tag="at", name=f"atd{c}{ib}{e}")
        nc.scalar.activation(atd, scd[e], Act.Exp)
        nc.vector.tensor_mul(
            atd.rearrange("p (i q) -> p i q", q=P),
            atd.rearrange("p (i q) -> p i q", q=P),
            st["m_diag"][:, None, :].to_broadcast((P, 4, P)),
        )
        for i4 in range(4):
            i = 4 * ib + i4
            nc.tensor.matmul(
                ctx[e][:, P * i4 : P * (i4 + 1)],
                st["v_xt"][:, i, vcol[e]],
                atd[:, P * i4 : P * (i4 + 1)],
                start=False,
                stop=(i4 == 3),
            )


def _norm_bank(nc, st, ctxT, c, half, e, bk, ctx_tile, tmp, scr):
    """ctxT rows = ctx[:64] * (1/denom) for one 512-wide q bank.
    denom = ctx row 64. Odd head (e=1) lands on partitions 64-127 of ctxT
    via an SBUF-to-SBUF DMA (engines cannot move data across partitions)."""
    span = slice(L * half + 512 * bk, L * half + 512 * (bk + 1))
    recip = tmp.tile([P, 512], F16, tag="rc", name=f"rc{c}{half}{e}{bk}")
    with nc.allow_low_precision(reason="fp16 recip"):
        nc.vector.reciprocal(recip[HD : HD + 1, :], ctx_tile[HD : HD + 1, :])
    bc = scr.tile([P, 512], F32, tag="ps", name=f"bc{c}{half}{e}{bk}")[:HD, :]
    nc.tensor.matmul(
        bc,
        st["ones_t"][HD : HD + 1, :],
        recip[HD : HD + 1, :],
        start=True,
        stop=True,
    )
    # DVE ops may read at most one PSUM operand: stage bc in SBUF
    rb = tmp.tile([HD, 512], F16, tag="rb", name=f"rb{c}{half}{e}{bk}")
    with nc.allow_low_precision(reason="fp16 ctx"):
        nc.vector.tensor_copy(rb, bc)
        if e == 0:
            nc.vector.tensor_mul(ctxT[:HD, c, span], ctx_tile[:HD, :], rb)
        else:
            cs = tmp.tile([HD, 512], F16, tag="cs", name=f"cs{c}{half}{bk}")
            nc.vector.tensor_mul(cs, ctx_tile[:HD, :], rb)
            nc.sync.dma_start(ctxT[HD : 2 * HD, c, span], cs)


def _masks():
    q = np.arange(P)[None, :] // BS
    k = np.arange(P)[:, None] // BS
    m = np.zeros((3, P, P), np.float16)
    m[0] = (q > k).astype(np.float16)    # strict (xt q vs x0 k, same tile)
    m[1] = (q >= k).astype(np.float16)   # incl (x0 q vs x0 k, same tile)
    m[2] = (q == k).astype(np.float16)   # diag (xt q vs xt k, same tile)
    return m


def _in_maps(x, Wq, bq, Wk, bk, Wv, Wo):
    masks = _masks()
    scale = HD ** -0.5
    in_maps = []
    for core in range(8):
        b, g = core // 2, core % 2
        cols = slice(DG * g, DG * (g + 1))
        in_maps.append(
            {
                "xT": np.ascontiguousarray(x[b].T).astype(np.float16),
                "wq": (np.ascontiguousarray(Wq[:, cols]) * np.float32(scale)).astype(
                    np.float16
                ),
                "wk": np.ascontiguousarray(Wk[:, cols]).astype(np.float16),
                "wv": np.ascontiguousarray(Wv[:, cols]).astype(np.float16),
                "wo": np.ascontiguousarray(Wo[cols, :]).astype(np.float16),
                "bqs": np.ascontiguousarray(bq[cols]).astype(np.float32)
                * np.float32(scale),
                "bks": np.ascontiguousarray(bk[cols]).astype(np.float32),
                "msk": masks,
            }
        )
    return in_maps


def kernel(x, Wq, bq, Wk, bk, Wv, bv, Wo, bo, block_size=4, **_):
    x = np.asarray(x, np.float32)
    Wq, bq = np.asarray(Wq, np.float32), np.asarray(bq, np.float32)
    Wk, bk = np.asarray(Wk, np.float32), np.asarray(bk, np.float32)
    Wv, bv = np.asarray(Wv, np.float32), np.asarray(bv, np.float32)
    Wo, bo = np.asarray(Wo, np.float32), np.asarray(bo, np.float32)

    if "nc" not in _CACHE:
        _CACHE["nc"] = _build()
    nc = _CACHE["nc"]

    in_maps = _in_maps(x, Wq, bq, Wk, bk, Wv, Wo)
    _CACHE["last_in_maps"] = in_maps
    last_err = None
    for _attempt in range(6):
        try:
            res = run_bass_kernel_spmd(nc, in_maps, core_ids=list(range(8)), trace=False)
            break
        except Exception as e:  # transient NRT device flakes
            last_err = e
            msg = str(e)
            if "UNRECOVERABLE" not in msg and "UNAVAILABLE" not in msg:
                raise
            import time as _time

            import jax as _jax

            _time.sleep(5 * (_attempt + 1))
            try:
                _jax.clear_backends()
            except Exception:
                pass
    else:
        raise last_err

    corr = (bv @ Wo + bo).astype(np.float32)  # softmax rows sum to 1
    out = np.empty((B, T, D), np.float32)
    for b in range(B):
        out[b] = res.results[2 * b]["out"] + res.results[2 * b + 1]["out"] + corr
    return out


if __name__ == "__main__":
    rng = np.random.default_rng(0)
    inputs = {
        "x": rng.standard_normal((B, T, D)).astype(np.float32),
        "Wq": (rng.standard_normal((D, D)) / 32).astype(np.float32),
        "bq": np.zeros(D, np.float32),
        "Wk": (rng.standard_normal((D, D)) / 32).astype(np.float32),
        "bk": np.zeros(D, np.float32),
        "Wv": (rng.standard_normal((D, D)) / 32).astype(np.float32),
        "bv": np.zeros(D, np.float32),
        "Wo": (rng.standard_normal((D, D)) / 32).astype(np.float32),
        "bo": np.zeros(D, np.float32),
    }
    o = kernel(**inputs)
    print("ran", o.shape, o.dtype, float(np.abs(o).max()))
